# revision 37
# baseline (speedup 1.0000x reference)
"""Trainium2 Bass kernel for nn_BertClassifier span-pair classifier.

Math (reference):
  vecs = hidden[:, 1:T+1, :]                                   [B,T,D]
  feat[b,i,j] = [vecs[b,i], vecs[b,j], ind[b,i,j]]             [2D+1]
  h   = relu(feat @ W1 + b1)                                   [B,T,T,H]
  out = h @ W2 + b2                                            [B,T,T,L]
  out = where(span_avail >= 1, out, 0)
  y   = log_softmax(out.reshape(B, T*T, L), axis=1)

Factorization used here (40x FLOP reduction over the naive 1537-wide GEMM):
  h[b,i,j] = relu(A[b,i] + C[b,j] + b1 + ind[b,i,j] * wlast)
  with A = vecs @ W1[:D], C = vecs @ W1[D:2D], wlast = W1[2D].

Sharding: 8 cores, core c = (b = c//2, parity p = c%2); core handles rows
i = p, p+2, ..., p+126 of batch b (parity striping keeps the SPMD program
identical across cores: the static suffix window for the span-indicator
correction of local slot ii is [2*ii, 128), which covers [i, 128) for both
parities, and the indicator is zero at j < i so the 1-column overshoot for
parity 1 is harmless).

Everything span-dependent is data (a [64,128] indicator grid built on-device
from iota + compares against shipped start/end scalars), so one program
serves all cores and all inputs; it is built and compiled exactly once.

H is padded 770->896 (7 chunks of 128); b1 is folded into the A-side bias
columns; b2 is added exactly (f32) to the GEMM psum during masking.
The second GEMM and the s-assembly run in bf16 (PE fp32 moving operands are
~4x slower); biases, psum accumulation, exp/LSE and the output stay f32.

log_softmax: per-core partial sums S_c[l] = sum_ij exp(val) (masked entries
contribute exp(0)=1), AllReduce-add over the 8 cores, LSE = ln(S), out = val
- LSE.  Values are O(+-8) so the max-free LSE is numerically safe in f32.
"""
import sys
from contextlib import ExitStack

sys.path.insert(0, "/opt/trn_rl_repo")

import numpy as np

import concourse.bass as bass
import concourse.tile as tile
from concourse import bacc, bass_utils, mybir
from concourse.masks import make_identity

B, T, D, H, L = 4, 128, 768, 770, 40
HP = 896            # H padded to 7*128
HC = HP // 128      # 7 h-chunks
DC = D // 128       # 6 d-chunks
IH = T // 2         # 64 local rows per core
N_CORES = 8
F32 = mybir.dt.float32
BF16 = mybir.dt.bfloat16
I32 = mybir.dt.int32
QUAD = 4            # i-rows per psum/batch group
_NQ_LIMIT = [None]  # dev knob: limit quads for timeline bisection
_EARLY_QUADS = [0]  # chunk-granular quads that overlap the first GEMM (0 = off; measured slower)
_RELU_CYCLE = ["pool"] * 11 + ["act"] * 10 + ["dve"] * 7


def _ap(ap_, dims, offset_elems=0):
    """Build an AP with explicit free-dim [step, count] pairs (step 0 = re-read)
    on top of ap_'s partition dim, offset in elements from ap_'s start."""
    import dataclasses
    return dataclasses.replace(
        ap_, ap=[ap_.ap[0]] + [list(d) for d in dims],
        offset=ap_.offset + offset_elems)

def build_program(timing_mode=False):
    """timing_mode=True builds a single-core variant with the AllReduce
    replaced by an equivalent local DRAM->DRAM copy, so the cost-model
    timeline simulator (which cannot model collectives) can run it."""
    nc = bacc.Bacc("TRN2", target_bir_lowering=False, debug=False,
                   num_devices=N_CORES)
    nc._timing_mode = timing_mode

    # ---- per-core I/O ----
    d_vecsf = nc.dram_tensor("vecs_full", [T, D], F32, kind="ExternalInput")
    d_vecsl = nc.dram_tensor("vecs_loc", [IH, D], F32, kind="ExternalInput")
    d_w1a = nc.dram_tensor("w1a", [D, HP], BF16, kind="ExternalInput")
    d_w1b = nc.dram_tensor("w1b", [D, HP], BF16, kind="ExternalInput")
    d_b1p = nc.dram_tensor("b1p", [HP], F32, kind="ExternalInput")
    d_wlp = nc.dram_tensor("wlp", [HP], F32, kind="ExternalInput")
    d_w2p = nc.dram_tensor("w2p", [HP, L], F32, kind="ExternalInput")
    d_b2 = nc.dram_tensor("b2", [L], F32, kind="ExternalInput")
    d_avail = nc.dram_tensor("avail", [IH, T], I32, kind="ExternalInput")
    d_meta = nc.dram_tensor("meta", [1, 8], F32, kind="ExternalInput")
    d_out = nc.dram_tensor("out", [IH * T, L], F32, kind="ExternalOutput")

    with tile.TileContext(nc) as tc, ExitStack() as stack:
        _build_tile(stack, tc, nc, d_vecsf, d_vecsl, d_w1a, d_w1b, d_b1p, d_wlp,
                    d_w2p, d_b2, d_avail, d_meta, d_out)
    nc.compile()
    return nc


def _build_tile(stack, tc, nc, d_vecsf, d_vecsl, d_w1a, d_w1b, d_b1p, d_wlp,
                d_w2p, d_b2, d_avail, d_meta, d_out):
    Relu = mybir.ActivationFunctionType
    Alu = mybir.AluOpType

    const = stack.enter_context(tc.tile_pool(name="const", bufs=1))
    persist = stack.enter_context(tc.tile_pool(name="persist", bufs=1))

    ident = const.tile([128, 128], F32)
    make_identity(nc, ident[:])

    # b1T / wlT column layouts: [128, HC] with [p, c] = vec[c*128+p]
    b1T = const.tile([128, HC], F32)
    nc.sync.dma_start(b1T[:], d_b1p.ap().rearrange("(c p) -> p c", p=128))
    wlT = const.tile([128, HC], F32)
    nc.sync.dma_start(wlT[:], d_wlp.ap().rearrange("(c p) -> p c", p=128))

    # W2 chunks as bf16 lhsT tiles [128, L] each
    w2f = const.tile([128, HC, L], F32)
    for c in range(HC):
        nc.sync.dma_start(w2f[:, c, :], d_w2p.ap()[c * 128:(c + 1) * 128, :])
    w2sb = const.tile([128, HC, L], BF16)
    nc.vector.tensor_copy(w2sb[:], w2f[:])
    wlTb = const.tile([128, HC], BF16)
    nc.vector.tensor_copy(wlTb[:], wlT[:])
    b2col = const.tile([L, 1], F32)
    nc.sync.dma_start(b2col[:], d_b2.ap().rearrange("(l a) -> l a", a=1))

    # ---- span indicator grid WROW [IH, 128] and mask columns ----
    meta1 = const.tile([1, 8], F32)
    nc.sync.dma_start(meta1[:], d_meta.ap())
    metab = const.tile([IH, 8], F32)
    nc.gpsimd.partition_broadcast(metab[:], meta1[:])
    scol = metab[:, 0:1]
    ecol = metab[:, 1:2]
    pcol = metab[:, 2:3]

    jrow_i = const.tile([IH, 128], I32)
    nc.gpsimd.iota(jrow_i[:], pattern=[[1, 128]], base=0, channel_multiplier=0)
    jrowf = const.tile([IH, 128], F32)
    nc.vector.tensor_copy(jrowf[:], jrow_i[:])
    gcol_i = const.tile([IH, 1], I32)
    nc.gpsimd.iota(gcol_i[:], pattern=[[0, 1]], base=0, channel_multiplier=2)
    gcolf0 = const.tile([IH, 1], F32)
    nc.vector.tensor_copy(gcolf0[:], gcol_i[:])
    gcolf = const.tile([IH, 1], F32)   # global row index i = 2*ii + p
    nc.vector.tensor_scalar(gcolf[:], gcolf0[:], pcol, None, Alu.add)

    c_jge = const.tile([IH, 128], F32)   # j >= i
    nc.vector.tensor_scalar(c_jge[:], jrowf[:], gcolf[:], None, Alu.is_ge)
    c_jle = const.tile([IH, 128], F32)   # j <= end
    nc.vector.tensor_scalar(c_jle[:], jrowf[:], ecol, None, Alu.is_le)
    band = const.tile([IH, 128], F32)
    nc.vector.tensor_tensor(band[:], c_jge[:], c_jle[:], Alu.mult)
    gin1 = const.tile([IH, 1], F32)      # i >= start
    nc.vector.tensor_scalar(gin1[:], gcolf[:], scol, None, Alu.is_ge)
    gin2 = const.tile([IH, 1], F32)      # i <= end
    nc.vector.tensor_scalar(gin2[:], gcolf[:], ecol, None, Alu.is_le)
    gin = const.tile([IH, 1], F32)
    nc.vector.tensor_tensor(gin[:], gin1[:], gin2[:], Alu.mult)
    wrow1 = const.tile([IH, 128], F32)
    nc.vector.tensor_scalar(wrow1[:], band[:], gin[:], None, Alu.mult)
    fg = const.tile([IH, 1], F32)        # i == start
    nc.vector.tensor_scalar(fg[:], gcolf[:], scol, None, Alu.is_equal)
    fj = const.tile([IH, 128], F32)      # j == end
    nc.vector.tensor_scalar(fj[:], jrowf[:], ecol, None, Alu.is_equal)
    fcell = const.tile([IH, 128], F32)
    nc.vector.tensor_scalar(fcell[:], fj[:], fg[:], None, Alu.mult)
    wrow = const.tile([IH, 128], F32)    # ind values in {0,1,2}
    nc.vector.tensor_tensor(wrow[:], wrow1[:], fcell[:], Alu.add)

    wrowB = const.tile([IH, 128], BF16)
    nc.vector.tensor_copy(wrowB[:], wrow[:])
    wstall = const.tile([1, IH * 128], BF16)   # all indicator rows on part 0
    nc.sync.dma_start(wstall[:].rearrange("a (i j) -> a i j", i=IH), wrowB[:])

    # avail rows as f32, staged to partition 0 for per-quad broadcasts
    availn = const.tile([IH, 128], I32)
    nc.sync.dma_start(availn[:], d_avail.ap())
    availf = const.tile([IH, 128], F32)
    nc.vector.tensor_copy(availf[:], availn[:])
    avstall = const.tile([1, IH * 128], F32)
    nc.sync.dma_start(avstall[:].rearrange("a (i j) -> a i j", i=IH), availf[:])

    # ---- first GEMM: AT(+b1) [128, HC, IH], CT [128, HC*128] ----
    ATb = persist.tile([128, HC, IH], F32)
    CT = persist.tile([128, HC * 128], BF16)

    dmae = [nc.sync, nc.scalar, nc.gpsimd]
    with tc.tile_pool(name="g1sbuf", bufs=1) as g1, \
         tc.tile_pool(name="g1psum", bufs=3, space="PSUM") as g1p, \
         tc.tile_pool(name="g1tp", bufs=2, space="PSUM") as g1tp:
        vf = g1.tile([T, D], F32)
        nc.sync.dma_start(vf[:], d_vecsf.ap())
        vl = g1.tile([IH, D], F32)
        nc.scalar.dma_start(vl[:], d_vecsl.ap())
        # W1 halves, bf16 on the wire, spread across engine DMA queues
        w1_sb = g1.tile([128, 2, DC, HP], BF16)
        for dc in range(DC):
            dmae[dc % 3].dma_start(w1_sb[:, 0, dc, :],
                                   d_w1a.ap()[dc * 128:(dc + 1) * 128, :])
            dmae[(dc + 1) % 3].dma_start(w1_sb[:, 1, dc, :],
                                         d_w1b.ap()[dc * 128:(dc + 1) * 128, :])

        # transposes of vecs into [d, i|j] layouts, cast to bf16
        # vT cols: [0:IH) = local i rows, [IH:IH+128) = full j rows
        vT = g1.tile([128, DC, IH + 128], BF16)
        for dc in range(DC):
            pt = g1tp.tile([128, 128], F32, tag='g1t')
            nc.tensor.transpose(pt[:], vf[:, dc * 128:(dc + 1) * 128],
                                ident[:])
            nc.scalar.copy(vT[:, dc, IH:], pt[:])
            pt2 = g1tp.tile([128, 128], F32, tag='g1t')
            nc.tensor.transpose(pt2[:, :IH], vl[:, dc * 128:(dc + 1) * 128],
                                ident[:IH, :IH])
            nc.scalar.copy(vT[:, dc, :IH], pt2[:, :IH])

        for hc in range(HC):
            pa = g1p.tile([128, IH + 128], F32, tag='g1mm')
            for dc in range(DC):
                nc.tensor.matmul(pa[:, :IH],
                                 w1_sb[:, 0, dc, hc * 128:(hc + 1) * 128],
                                 vT[:, dc, :IH], start=(dc == 0),
                                 stop=(dc == DC - 1))
            for dc in range(DC):
                nc.tensor.matmul(pa[:, IH:],
                                 w1_sb[:, 1, dc, hc * 128:(hc + 1) * 128],
                                 vT[:, dc, IH:], start=(dc == 0),
                                 stop=(dc == DC - 1))
            nc.vector.tensor_scalar(ATb[:, hc, :], pa[:, :IH], b1T[:, hc:hc + 1],
                                    None, Alu.add)
            nc.scalar.copy(CT[:, hc * 128:(hc + 1) * 128], pa[:, IH:])

    # ---- main loop over local rows, quads of 4 ----
    valT = persist.tile([128, IH * L], F32)
    Scols = persist.tile([L, IH // QUAD], F32)

    windp = stack.enter_context(tc.tile_pool(name="windsb", bufs=5))
    m40p = stack.enter_context(tc.tile_pool(name="m40", bufs=5))
    stp = stack.enter_context(tc.tile_pool(name="st", bufs=5))
    tmpp = stack.enter_context(tc.tile_pool(name="tmp", bufs=3))
    s1p = stack.enter_context(tc.tile_pool(name="s1", bufs=3))
    v40p = stack.enter_context(tc.tile_pool(name="v40", bufs=3))
    gp = stack.enter_context(tc.tile_pool(name="gpsum", bufs=3, space="PSUM"))
    tpp = stack.enter_context(tc.tile_pool(name="tpsum", bufs=2, space="PSUM"))

    # relu engine split per (k, c) slot: mostly pool/act, some dve
    _RELU = {"dve": nc.vector, "act": None, "pool": nc.gpsimd}
    relu_cycle = list(_RELU_CYCLE)
    assert len(relu_cycle) == QUAD * HC

    n_q = IH // QUAD
    if _NQ_LIMIT[0] is not None:
        n_q = _NQ_LIMIT[0]
    for q in range(n_q):
        s = 2 * QUAD * q            # uniform suffix start for the quad
        w = 128 - s

        wind = windp.tile([128, QUAD * 128], BF16, tag="wind")
        nc.gpsimd.partition_broadcast(
            wind[:], wstall[:, q * QUAD * 128:(q + 1) * QUAD * 128])
        mask40 = m40p.tile([L, QUAD * 128], F32, tag="m40")
        nc.gpsimd.partition_broadcast(
            mask40[:], avstall[:, q * QUAD * 128:(q + 1) * QUAD * 128])

        st = stp.tile([128, QUAD, HC * 128], BF16, tag="st")
        tmp = tmpp.tile([128, QUAD * HC * w], BF16, tag="tmp")

        if q < _EARLY_QUADS[0]:
            # chunk-granular build: each chunk gates only on its own
            # CT/ATb slice, so these quads overlap the first GEMM's tail
            for c in range(HC):
                nc.vector.tensor_tensor(
                    _ap(tmp[:], [[HC * w, QUAD], [1, w]], offset_elems=c * w),
                    _ap(wind[:], [[128, QUAD], [1, w]], offset_elems=s),
                    _ap(wlTb[:], [[0, QUAD], [0, w]], offset_elems=c),
                    Alu.mult)
                nc.vector.tensor_tensor(
                    _ap(st[:], [[HC * 128, QUAD], [1, w]],
                        offset_elems=c * 128 + s),
                    _ap(CT[:], [[0, QUAD], [1, w]], offset_elems=c * 128 + s),
                    _ap(tmp[:], [[HC * w, QUAD], [1, w]], offset_elems=c * w),
                    Alu.add)
                if s > 0:
                    nc.vector.tensor_copy(
                        _ap(st[:], [[HC * 128, QUAD], [1, s]],
                            offset_elems=c * 128),
                        _ap(CT[:], [[0, QUAD], [1, s]], offset_elems=c * 128))
        else:
            # tmp[p,(k,c,j)] = wind[p,(k,j+s)] * wlT[p,c]     (one DVE op)
            nc.vector.tensor_tensor(
                _ap(tmp[:], [[HC * w, QUAD], [w, HC], [1, w]]),
                _ap(wind[:], [[128, QUAD], [0, HC], [1, w]], offset_elems=s),
                _ap(wlTb[:], [[0, QUAD], [1, HC], [0, w]]),
                Alu.mult)
            # st suffix = CT + tmp                            (one DVE op)
            nc.vector.tensor_tensor(
                _ap(st[:], [[HC * 128, QUAD], [128, HC], [1, w]],
                    offset_elems=s),
                _ap(CT[:], [[0, QUAD], [128, HC], [1, w]], offset_elems=s),
                _ap(tmp[:], [[HC * w, QUAD], [w, HC], [1, w]]),
                Alu.add)
            # st prefix = CT (uncorrected region)             (one DVE op)
            if s > 0:
                nc.vector.tensor_copy(
                    _ap(st[:], [[HC * 128, QUAD], [128, HC], [1, s]]),
                    _ap(CT[:], [[0, QUAD], [128, HC], [1, s]]))

        # relu in place with per-(i,chunk) bias
        for k in range(QUAD):
            ii = q * QUAD + k
            for c in range(HC):
                eng = relu_cycle[k * HC + c]
                tgt = st[:, k, c * 128:(c + 1) * 128]
                bias = ATb[:, c, ii:ii + 1]
                if eng == "act":
                    nc.scalar.activation(tgt, tgt,
                                         mybir.ActivationFunctionType.Relu,
                                         bias=bias)
                else:
                    _RELU[eng].tensor_scalar(tgt, tgt, bias, 0.0,
                                             Alu.add, Alu.max)

        # second GEMM: psum[l, (k,j)] += W2c.T @ st[:, :, c]   N=512 bf16
        gpsum = gp.tile([L, QUAD * 128], F32, tag="gp")
        for c in range(HC):
            nc.tensor.matmul(
                gpsum[:],
                w2sb[:, c, :],
                _ap(st[:], [[HC * 128, QUAD], [1, 128]], offset_elems=c * 128),
                start=(c == 0), stop=(c == HC - 1))

        # val40 = (psum + b2) * mask;  exp-accum -> Scols[:, q]
        v40 = v40p.tile([L, QUAD * 128], F32, tag="v40")
        nc.vector.scalar_tensor_tensor(v40[:], gpsum[:], b2col[:], mask40[:],
                                       Alu.add, Alu.mult)
        scr = s1p.tile([L, QUAD * 128], F32, tag="s1")
        nc.scalar.activation(scr[:], v40[:], Relu.Exp,
                             accum_out=Scols[:, q:q + 1])

        # transpose to [128(j), 40] and store into valT
        tp4 = tpp.tile([128, QUAD, L], F32, tag="tp")
        for k in range(QUAD):
            nc.tensor.transpose(tp4[:, k, :], v40[:, k * 128:(k + 1) * 128],
                                ident[:L, :L])
        nc.vector.tensor_copy(valT[:, q * QUAD * L:(q + 1) * QUAD * L],
                              tp4[:])

    # ---- AllReduce of exp-sums, LSE, subtract, store ----
    S_col = persist.tile([L, 1], F32)
    nc.vector.tensor_reduce(S_col[:], Scols[:], mybir.AxisListType.X, Alu.add)
    # to a [1, L] row via PE transpose
    with tc.tile_pool(name="sps", bufs=1, space="PSUM") as sps:
        spt = sps.tile([1, L], F32)
        nc.tensor.transpose(spt[:], S_col[:], ident[:L, :L])
        S_sb = persist.tile([1, L], F32)
        nc.scalar.copy(S_sb[:], spt[:])
    with tc.tile_pool(name="dram", bufs=1, space="DRAM") as dram:
        cin = dram.tile([1, L], F32)
        cout = dram.tile([1, L], F32)
        nc.sync.dma_start(cin[:], S_sb[:])
        if getattr(nc, "_timing_mode", False):
            nc.sync.dma_start(cout[:], cin[:])
        else:
            nc.gpsimd.collective_compute(
                "AllReduce", Alu.add,
                replica_groups=[[2 * b, 2 * b + 1] for b in range(B)],
                ins=[cin.opt()], outs=[cout.opt()],
            )
        S_row = persist.tile([1, L], F32)
        nc.sync.dma_start(S_row[:], cout[:])

    lse0 = persist.tile([128, L], F32)
    nc.gpsimd.partition_broadcast(lse0[:], S_row[:])
    lse = persist.tile([128, L], F32)
    nc.scalar.activation(lse[:], lse0[:], Relu.Ln)

    outf = persist.tile([128, IH * L], F32)
    out3 = d_out.ap().rearrange("(i j) l -> j i l", j=128)
    outf3 = outf[:].rearrange("p (i l) -> p i l", i=IH)
    CH = 16
    dmas = [nc.sync, nc.scalar]
    for t in range(IH // CH):
        lo, hi = t * CH, (t + 1) * CH
        nc.vector.tensor_tensor(
            _ap(outf[:], [[L, CH], [1, L]], offset_elems=lo * L),
            _ap(valT[:], [[L, CH], [1, L]], offset_elems=lo * L),
            _ap(lse[:], [[0, CH], [1, L]]),
            Alu.subtract)
        dmas[t % 2].dma_start(out3[:, lo:hi, :], outf3[:, lo:hi, :])


_NC_CACHE = {}


def _get_program():
    if "nc" not in _NC_CACHE:
        _NC_CACHE["nc"] = build_program()
    return _NC_CACHE["nc"]


def make_in_maps(hidden, W1, b1, W2, b2, pred_spans, span_avail):
    """Build the 8 per-core input dicts (all numpy, f32/i32)."""
    hidden = np.asarray(hidden, np.float32)
    W1 = np.asarray(W1, np.float32)
    b1 = np.asarray(b1, np.float32)
    W2 = np.asarray(W2, np.float32)
    b2 = np.asarray(b2, np.float32)
    pred_spans = np.asarray(pred_spans).astype(np.int64)
    span_avail = np.asarray(span_avail).astype(np.int32)

    vecs = hidden[:, 1:T + 1, :]                      # [B,T,D]
    import ml_dtypes
    w1a = np.zeros((D, HP), ml_dtypes.bfloat16)
    w1a[:, :H] = W1[:D].astype(ml_dtypes.bfloat16)
    w1b = np.zeros((D, HP), ml_dtypes.bfloat16)
    w1b[:, :H] = W1[D:2 * D].astype(ml_dtypes.bfloat16)
    b1p = np.zeros((HP,), np.float32)
    b1p[:H] = b1
    wlp = np.zeros((HP,), np.float32)
    wlp[:H] = W1[2 * D]
    w2p = np.zeros((HP, L), np.float32)
    w2p[:H] = W2

    in_maps = []
    for c in range(N_CORES):
        b, p = c // 2, c % 2
        meta = np.zeros((1, 8), np.float32)
        meta[0, 0] = float(pred_spans[b, 0])
        meta[0, 1] = float(pred_spans[b, 1])
        meta[0, 2] = float(p)
        in_maps.append({
            "vecs_full": np.ascontiguousarray(vecs[b]),
            "vecs_loc": np.ascontiguousarray(vecs[b, p::2]),
            "w1a": w1a, "w1b": w1b, "b1p": b1p, "wlp": wlp, "w2p": w2p,
            "b2": b2,
            "avail": np.ascontiguousarray(span_avail[p::2]),
            "meta": meta,
        })
    return in_maps


def unshard(results):
    """results: list of 8 dicts with 'out' [IH*T, L] -> full [B, T*T, L]."""
    full = np.empty((B, T, T, L), np.float32)
    for c in range(N_CORES):
        b, p = c // 2, c % 2
        full[b, p::2] = results[c]["out"].reshape(IH, T, L)
    return full.reshape(B, T * T, L)


def kernel(hidden, W1, b1, W2, b2, pred_spans, span_avail, token_num):
    assert int(np.asarray(token_num)) == T, "kernel specialized for T=128"
    in_maps = make_in_maps(hidden, W1, b1, W2, b2, pred_spans, span_avail)
    nc = _get_program()
    res = bass_utils.run_bass_kernel_spmd(
        nc, in_maps, core_ids=list(range(N_CORES)))
    return unshard(res.results)


# revision 41
# speedup vs baseline: 1.0133x; 1.0133x over previous
"""Trainium2 Bass kernel for nn_BertClassifier span-pair classifier.

Math (reference):
  vecs = hidden[:, 1:T+1, :]                                   [B,T,D]
  feat[b,i,j] = [vecs[b,i], vecs[b,j], ind[b,i,j]]             [2D+1]
  h   = relu(feat @ W1 + b1)                                   [B,T,T,H]
  out = h @ W2 + b2                                            [B,T,T,L]
  out = where(span_avail >= 1, out, 0)
  y   = log_softmax(out.reshape(B, T*T, L), axis=1)

Factorization used here (40x FLOP reduction over the naive 1537-wide GEMM):
  h[b,i,j] = relu(A[b,i] + C[b,j] + b1 + ind[b,i,j] * wlast)
  with A = vecs @ W1[:D], C = vecs @ W1[D:2D], wlast = W1[2D].

Sharding: 8 cores, core c = (b = c//2, parity p = c%2); core handles rows
i = p, p+2, ..., p+126 of batch b (parity striping keeps the SPMD program
identical across cores: the static suffix window for the span-indicator
correction of local slot ii is [2*ii, 128), which covers [i, 128) for both
parities, and the indicator is zero at j < i so the 1-column overshoot for
parity 1 is harmless).

Everything span-dependent is data (a [64,128] indicator grid built on-device
from iota + compares against shipped start/end scalars), so one program
serves all cores and all inputs; it is built and compiled exactly once.

H is padded 770->896 (7 chunks of 128); b1 is folded into the A-side bias
columns; b2 is added exactly (f32) to the GEMM psum during masking.
The second GEMM and the s-assembly run in bf16 (PE fp32 moving operands are
~4x slower); biases, psum accumulation, exp/LSE and the output stay f32.

log_softmax: per-core partial sums S_c[l] = sum_ij exp(val) (masked entries
contribute exp(0)=1), AllReduce-add over the 8 cores, LSE = ln(S), out = val
- LSE.  Values are O(+-8) so the max-free LSE is numerically safe in f32.
"""
import sys
from contextlib import ExitStack

sys.path.insert(0, "/opt/trn_rl_repo")

import numpy as np

import concourse.bass as bass
import concourse.tile as tile
from concourse import bacc, bass_utils, mybir
from concourse.masks import make_identity

B, T, D, H, L = 4, 128, 768, 770, 40
HP = 896            # H padded to 7*128
HC = HP // 128      # 7 h-chunks
DC = D // 128       # 6 d-chunks
IH = T // 2         # 64 local rows per core
N_CORES = 8
F32 = mybir.dt.float32
BF16 = mybir.dt.bfloat16
I32 = mybir.dt.int32
QUAD = 4            # i-rows per psum/batch group
_NQ_LIMIT = [None]  # dev knob: limit quads for timeline bisection
_EARLY_QUADS = [0]  # chunk-granular quads that overlap the first GEMM (0 = off; measured slower)
_RELU_CYCLE = ["pool"] * 11 + ["act"] * 9 + ["dve"] * 8


def _ap(ap_, dims, offset_elems=0):
    """Build an AP with explicit free-dim [step, count] pairs (step 0 = re-read)
    on top of ap_'s partition dim, offset in elements from ap_'s start."""
    import dataclasses
    return dataclasses.replace(
        ap_, ap=[ap_.ap[0]] + [list(d) for d in dims],
        offset=ap_.offset + offset_elems)

def build_program(timing_mode=False):
    """timing_mode=True builds a single-core variant with the AllReduce
    replaced by an equivalent local DRAM->DRAM copy, so the cost-model
    timeline simulator (which cannot model collectives) can run it."""
    nc = bacc.Bacc("TRN2", target_bir_lowering=False, debug=False,
                   num_devices=N_CORES)
    nc._timing_mode = timing_mode

    # ---- per-core I/O ----
    d_vecsf = nc.dram_tensor("vecs_full", [T, D], F32, kind="ExternalInput")
    d_vecsl = nc.dram_tensor("vecs_loc", [IH, D], F32, kind="ExternalInput")
    d_w1a = nc.dram_tensor("w1a", [D, HP], BF16, kind="ExternalInput")
    d_w1b = nc.dram_tensor("w1b", [D, HP], BF16, kind="ExternalInput")
    d_b1p = nc.dram_tensor("b1p", [HP], F32, kind="ExternalInput")
    d_wlp = nc.dram_tensor("wlp", [HP], F32, kind="ExternalInput")
    d_w2p = nc.dram_tensor("w2p", [HP, L], F32, kind="ExternalInput")
    d_b2 = nc.dram_tensor("b2", [L], F32, kind="ExternalInput")
    d_avail = nc.dram_tensor("avail", [IH, T], I32, kind="ExternalInput")
    d_meta = nc.dram_tensor("meta", [1, 8], F32, kind="ExternalInput")
    d_out = nc.dram_tensor("out", [IH * T, L], F32, kind="ExternalOutput")

    with tile.TileContext(nc) as tc, ExitStack() as stack:
        _build_tile(stack, tc, nc, d_vecsf, d_vecsl, d_w1a, d_w1b, d_b1p, d_wlp,
                    d_w2p, d_b2, d_avail, d_meta, d_out)
    nc.compile()
    return nc


def _build_tile(stack, tc, nc, d_vecsf, d_vecsl, d_w1a, d_w1b, d_b1p, d_wlp,
                d_w2p, d_b2, d_avail, d_meta, d_out):
    Relu = mybir.ActivationFunctionType
    Alu = mybir.AluOpType

    const = stack.enter_context(tc.tile_pool(name="const", bufs=1))
    persist = stack.enter_context(tc.tile_pool(name="persist", bufs=1))

    ident = const.tile([128, 128], F32)
    make_identity(nc, ident[:])

    # b1T / wlT column layouts: [128, HC] with [p, c] = vec[c*128+p]
    b1T = const.tile([128, HC], F32)
    nc.sync.dma_start(b1T[:], d_b1p.ap().rearrange("(c p) -> p c", p=128))
    wlT = const.tile([128, HC], F32)
    nc.sync.dma_start(wlT[:], d_wlp.ap().rearrange("(c p) -> p c", p=128))

    # W2 chunks as bf16 lhsT tiles [128, L] each
    w2f = const.tile([128, HC, L], F32)
    for c in range(HC):
        nc.sync.dma_start(w2f[:, c, :], d_w2p.ap()[c * 128:(c + 1) * 128, :])
    w2sb = const.tile([128, HC, L], BF16)
    nc.vector.tensor_copy(w2sb[:], w2f[:])
    wlTb = const.tile([128, HC], BF16)
    nc.vector.tensor_copy(wlTb[:], wlT[:])
    b2col = const.tile([L, 1], F32)
    nc.sync.dma_start(b2col[:], d_b2.ap().rearrange("(l a) -> l a", a=1))

    # ---- span indicator grid WROW [IH, 128] and mask columns ----
    meta1 = const.tile([1, 8], F32)
    nc.sync.dma_start(meta1[:], d_meta.ap())
    metab = const.tile([IH, 8], F32)
    nc.gpsimd.partition_broadcast(metab[:], meta1[:])
    scol = metab[:, 0:1]
    ecol = metab[:, 1:2]
    pcol = metab[:, 2:3]

    jrow_i = const.tile([IH, 128], I32)
    nc.gpsimd.iota(jrow_i[:], pattern=[[1, 128]], base=0, channel_multiplier=0)
    jrowf = const.tile([IH, 128], F32)
    nc.vector.tensor_copy(jrowf[:], jrow_i[:])
    gcol_i = const.tile([IH, 1], I32)
    nc.gpsimd.iota(gcol_i[:], pattern=[[0, 1]], base=0, channel_multiplier=2)
    gcolf0 = const.tile([IH, 1], F32)
    nc.vector.tensor_copy(gcolf0[:], gcol_i[:])
    gcolf = const.tile([IH, 1], F32)   # global row index i = 2*ii + p
    nc.vector.tensor_scalar(gcolf[:], gcolf0[:], pcol, None, Alu.add)

    c_jge = const.tile([IH, 128], F32)   # j >= i
    nc.vector.tensor_scalar(c_jge[:], jrowf[:], gcolf[:], None, Alu.is_ge)
    c_jle = const.tile([IH, 128], F32)   # j <= end
    nc.vector.tensor_scalar(c_jle[:], jrowf[:], ecol, None, Alu.is_le)
    band = const.tile([IH, 128], F32)
    nc.vector.tensor_tensor(band[:], c_jge[:], c_jle[:], Alu.mult)
    gin1 = const.tile([IH, 1], F32)      # i >= start
    nc.vector.tensor_scalar(gin1[:], gcolf[:], scol, None, Alu.is_ge)
    gin2 = const.tile([IH, 1], F32)      # i <= end
    nc.vector.tensor_scalar(gin2[:], gcolf[:], ecol, None, Alu.is_le)
    gin = const.tile([IH, 1], F32)
    nc.vector.tensor_tensor(gin[:], gin1[:], gin2[:], Alu.mult)
    wrow1 = const.tile([IH, 128], F32)
    nc.vector.tensor_scalar(wrow1[:], band[:], gin[:], None, Alu.mult)
    fg = const.tile([IH, 1], F32)        # i == start
    nc.vector.tensor_scalar(fg[:], gcolf[:], scol, None, Alu.is_equal)
    fj = const.tile([IH, 128], F32)      # j == end
    nc.vector.tensor_scalar(fj[:], jrowf[:], ecol, None, Alu.is_equal)
    fcell = const.tile([IH, 128], F32)
    nc.vector.tensor_scalar(fcell[:], fj[:], fg[:], None, Alu.mult)
    wrow = const.tile([IH, 128], F32)    # ind values in {0,1,2}
    nc.vector.tensor_tensor(wrow[:], wrow1[:], fcell[:], Alu.add)

    wrowB = const.tile([IH, 128], BF16)
    nc.vector.tensor_copy(wrowB[:], wrow[:])
    wstall = const.tile([1, IH * 128], BF16)   # all indicator rows on part 0
    nc.sync.dma_start(wstall[:].rearrange("a (i j) -> a i j", i=IH), wrowB[:])

    # avail rows as f32, staged to partition 0 for per-quad broadcasts
    availn = const.tile([IH, 128], I32)
    nc.sync.dma_start(availn[:], d_avail.ap())
    availf = const.tile([IH, 128], F32)
    nc.vector.tensor_copy(availf[:], availn[:])
    avstall = const.tile([1, IH * 128], F32)
    nc.sync.dma_start(avstall[:].rearrange("a (i j) -> a i j", i=IH), availf[:])

    # ---- first GEMM: AT(+b1) [128, HC, IH], CT [128, HC*128] ----
    ATb = persist.tile([128, HC, IH], F32)
    CT = persist.tile([128, HC * 128], BF16)

    dmae = [nc.sync, nc.scalar, nc.gpsimd]
    with tc.tile_pool(name="g1sbuf", bufs=1) as g1, \
         tc.tile_pool(name="g1psum", bufs=3, space="PSUM") as g1p, \
         tc.tile_pool(name="g1tp", bufs=2, space="PSUM") as g1tp:
        vf = g1.tile([T, D], F32)
        nc.sync.dma_start(vf[:], d_vecsf.ap())
        vl = g1.tile([IH, D], F32)
        nc.scalar.dma_start(vl[:], d_vecsl.ap())
        # W1 halves, bf16 on the wire, spread across engine DMA queues
        w1_sb = g1.tile([128, 2, DC, HP], BF16)
        for dc in range(DC):
            dmae[dc % 3].dma_start(w1_sb[:, 0, dc, :],
                                   d_w1a.ap()[dc * 128:(dc + 1) * 128, :])
            dmae[(dc + 1) % 3].dma_start(w1_sb[:, 1, dc, :],
                                         d_w1b.ap()[dc * 128:(dc + 1) * 128, :])

        # transposes of vecs into [d, i|j] layouts, cast to bf16
        # vT cols: [0:IH) = local i rows, [IH:IH+128) = full j rows
        vT = g1.tile([128, DC, IH + 128], BF16)
        for dc in range(DC):
            pt = g1tp.tile([128, 128], F32, tag='g1t')
            nc.tensor.transpose(pt[:], vf[:, dc * 128:(dc + 1) * 128],
                                ident[:])
            nc.scalar.copy(vT[:, dc, IH:], pt[:])
            pt2 = g1tp.tile([128, 128], F32, tag='g1t')
            nc.tensor.transpose(pt2[:, :IH], vl[:, dc * 128:(dc + 1) * 128],
                                ident[:IH, :IH])
            nc.scalar.copy(vT[:, dc, :IH], pt2[:, :IH])

        for hc in range(HC):
            pa = g1p.tile([128, IH + 128], F32, tag='g1mm')
            for dc in range(DC):
                nc.tensor.matmul(pa[:, :IH],
                                 w1_sb[:, 0, dc, hc * 128:(hc + 1) * 128],
                                 vT[:, dc, :IH], start=(dc == 0),
                                 stop=(dc == DC - 1))
            for dc in range(DC):
                nc.tensor.matmul(pa[:, IH:],
                                 w1_sb[:, 1, dc, hc * 128:(hc + 1) * 128],
                                 vT[:, dc, IH:], start=(dc == 0),
                                 stop=(dc == DC - 1))
            nc.vector.tensor_scalar(ATb[:, hc, :], pa[:, :IH], b1T[:, hc:hc + 1],
                                    None, Alu.add)
            nc.scalar.copy(CT[:, hc * 128:(hc + 1) * 128], pa[:, IH:])

    # ---- main loop over local rows, quads of 4 ----
    valT = persist.tile([128, IH * L], F32)
    Scols = persist.tile([L, IH // QUAD], F32)

    windp = stack.enter_context(tc.tile_pool(name="windsb", bufs=5))
    m40p = stack.enter_context(tc.tile_pool(name="m40", bufs=5))
    stp = stack.enter_context(tc.tile_pool(name="st", bufs=5))
    tmpp = stack.enter_context(tc.tile_pool(name="tmp", bufs=3))
    s1p = stack.enter_context(tc.tile_pool(name="s1", bufs=3))
    v40p = stack.enter_context(tc.tile_pool(name="v40", bufs=3))
    gp = stack.enter_context(tc.tile_pool(name="gpsum", bufs=3, space="PSUM"))
    tpp = stack.enter_context(tc.tile_pool(name="tpsum", bufs=2, space="PSUM"))

    # relu engine split per (k, c) slot: mostly pool/act, some dve
    _RELU = {"dve": nc.vector, "act": None, "pool": nc.gpsimd}
    relu_cycle = list(_RELU_CYCLE)
    assert len(relu_cycle) == QUAD * HC

    n_q = IH // QUAD
    if _NQ_LIMIT[0] is not None:
        n_q = _NQ_LIMIT[0]
    for q in range(n_q):
        s = 2 * QUAD * q            # uniform suffix start for the quad
        w = 128 - s

        wind = windp.tile([128, QUAD * 128], BF16, tag="wind")
        nc.gpsimd.partition_broadcast(
            wind[:], wstall[:, q * QUAD * 128:(q + 1) * QUAD * 128])
        mask40 = m40p.tile([L, QUAD * 128], F32, tag="m40")
        nc.gpsimd.partition_broadcast(
            mask40[:], avstall[:, q * QUAD * 128:(q + 1) * QUAD * 128])

        st = stp.tile([128, QUAD, HC * 128], BF16, tag="st")
        tmp = tmpp.tile([128, QUAD * HC * w], BF16, tag="tmp")

        if q < _EARLY_QUADS[0]:
            # chunk-granular build: each chunk gates only on its own
            # CT/ATb slice, so these quads overlap the first GEMM's tail
            for c in range(HC):
                nc.vector.tensor_tensor(
                    _ap(tmp[:], [[HC * w, QUAD], [1, w]], offset_elems=c * w),
                    _ap(wind[:], [[128, QUAD], [1, w]], offset_elems=s),
                    _ap(wlTb[:], [[0, QUAD], [0, w]], offset_elems=c),
                    Alu.mult)
                nc.vector.tensor_tensor(
                    _ap(st[:], [[HC * 128, QUAD], [1, w]],
                        offset_elems=c * 128 + s),
                    _ap(CT[:], [[0, QUAD], [1, w]], offset_elems=c * 128 + s),
                    _ap(tmp[:], [[HC * w, QUAD], [1, w]], offset_elems=c * w),
                    Alu.add)
                if s > 0:
                    nc.vector.tensor_copy(
                        _ap(st[:], [[HC * 128, QUAD], [1, s]],
                            offset_elems=c * 128),
                        _ap(CT[:], [[0, QUAD], [1, s]], offset_elems=c * 128))
        else:
            # tmp[p,(k,c,j)] = wind[p,(k,j+s)] * wlT[p,c]     (one DVE op)
            nc.vector.tensor_tensor(
                _ap(tmp[:], [[HC * w, QUAD], [w, HC], [1, w]]),
                _ap(wind[:], [[128, QUAD], [0, HC], [1, w]], offset_elems=s),
                _ap(wlTb[:], [[0, QUAD], [1, HC], [0, w]]),
                Alu.mult)
            # st suffix = CT + tmp                            (one DVE op)
            nc.vector.tensor_tensor(
                _ap(st[:], [[HC * 128, QUAD], [128, HC], [1, w]],
                    offset_elems=s),
                _ap(CT[:], [[0, QUAD], [128, HC], [1, w]], offset_elems=s),
                _ap(tmp[:], [[HC * w, QUAD], [w, HC], [1, w]]),
                Alu.add)
            # st prefix = CT (uncorrected region)             (one DVE op)
            if s > 0:
                nc.vector.tensor_copy(
                    _ap(st[:], [[HC * 128, QUAD], [128, HC], [1, s]]),
                    _ap(CT[:], [[0, QUAD], [128, HC], [1, s]]))

        # relu in place with per-(i,chunk) bias
        for k in range(QUAD):
            ii = q * QUAD + k
            for c in range(HC):
                eng = relu_cycle[k * HC + c]
                tgt = st[:, k, c * 128:(c + 1) * 128]
                bias = ATb[:, c, ii:ii + 1]
                if eng == "act":
                    nc.scalar.activation(tgt, tgt,
                                         mybir.ActivationFunctionType.Relu,
                                         bias=bias)
                else:
                    _RELU[eng].tensor_scalar(tgt, tgt, bias, 0.0,
                                             Alu.add, Alu.max)

        # second GEMM: psum[l, (k,j)] += W2c.T @ st[:, :, c]   N=512 bf16
        gpsum = gp.tile([L, QUAD * 128], F32, tag="gp")
        for c in range(HC):
            nc.tensor.matmul(
                gpsum[:],
                w2sb[:, c, :],
                _ap(st[:], [[HC * 128, QUAD], [1, 128]], offset_elems=c * 128),
                start=(c == 0), stop=(c == HC - 1))

        # val40 = (psum + b2) * mask;  exp-accum -> Scols[:, q]
        v40 = v40p.tile([L, QUAD * 128], F32, tag="v40")
        nc.vector.scalar_tensor_tensor(v40[:], gpsum[:], b2col[:], mask40[:],
                                       Alu.add, Alu.mult)
        scr = s1p.tile([L, QUAD * 128], F32, tag="s1")
        nc.scalar.activation(scr[:], v40[:], Relu.Exp,
                             accum_out=Scols[:, q:q + 1])

        # transpose to [128(j), 40] and store into valT
        tp4 = tpp.tile([128, QUAD, L], F32, tag="tp")
        for k in range(QUAD):
            nc.tensor.transpose(tp4[:, k, :], v40[:, k * 128:(k + 1) * 128],
                                ident[:L, :L])
        nc.scalar.copy(valT[:, q * QUAD * L:(q + 1) * QUAD * L], tp4[:])

    # ---- AllReduce of exp-sums, LSE, subtract, store ----
    S_col = persist.tile([L, 1], F32)
    nc.vector.tensor_reduce(S_col[:], Scols[:], mybir.AxisListType.X, Alu.add)
    # to a [1, L] row via PE transpose
    with tc.tile_pool(name="sps", bufs=1, space="PSUM") as sps:
        spt = sps.tile([1, L], F32)
        nc.tensor.transpose(spt[:], S_col[:], ident[:L, :L])
        S_sb = persist.tile([1, L], F32)
        nc.scalar.copy(S_sb[:], spt[:])
    with tc.tile_pool(name="dram", bufs=1, space="DRAM") as dram:
        cin = dram.tile([1, L], F32)
        cout = dram.tile([1, L], F32)
        nc.sync.dma_start(cin[:], S_sb[:])
        if getattr(nc, "_timing_mode", False):
            nc.sync.dma_start(cout[:], cin[:])
        else:
            nc.gpsimd.collective_compute(
                "AllReduce", Alu.add,
                replica_groups=[[2 * b, 2 * b + 1] for b in range(B)],
                ins=[cin.opt()], outs=[cout.opt()],
            )
        S_row = persist.tile([1, L], F32)
        nc.sync.dma_start(S_row[:], cout[:])

    lse0 = persist.tile([128, L], F32)
    nc.gpsimd.partition_broadcast(lse0[:], S_row[:])
    lse = persist.tile([128, L], F32)
    nc.scalar.activation(lse[:], lse0[:], Relu.Ln)

    outf = persist.tile([128, IH * L], F32)
    out3 = d_out.ap().rearrange("(i j) l -> j i l", j=128)
    outf3 = outf[:].rearrange("p (i l) -> p i l", i=IH)
    CH = 16
    dmas = [nc.sync, nc.scalar]
    for t in range(IH // CH):
        lo, hi = t * CH, (t + 1) * CH
        nc.vector.tensor_tensor(
            _ap(outf[:], [[L, CH], [1, L]], offset_elems=lo * L),
            _ap(valT[:], [[L, CH], [1, L]], offset_elems=lo * L),
            _ap(lse[:], [[0, CH], [1, L]]),
            Alu.subtract)
        dmas[t % 2].dma_start(out3[:, lo:hi, :], outf3[:, lo:hi, :])


_NC_CACHE = {}


def _get_program():
    if "nc" not in _NC_CACHE:
        _NC_CACHE["nc"] = build_program()
    return _NC_CACHE["nc"]


def make_in_maps(hidden, W1, b1, W2, b2, pred_spans, span_avail):
    """Build the 8 per-core input dicts (all numpy, f32/i32)."""
    hidden = np.asarray(hidden, np.float32)
    W1 = np.asarray(W1, np.float32)
    b1 = np.asarray(b1, np.float32)
    W2 = np.asarray(W2, np.float32)
    b2 = np.asarray(b2, np.float32)
    pred_spans = np.asarray(pred_spans).astype(np.int64)
    span_avail = np.asarray(span_avail).astype(np.int32)

    vecs = hidden[:, 1:T + 1, :]                      # [B,T,D]
    import ml_dtypes
    w1a = np.zeros((D, HP), ml_dtypes.bfloat16)
    w1a[:, :H] = W1[:D].astype(ml_dtypes.bfloat16)
    w1b = np.zeros((D, HP), ml_dtypes.bfloat16)
    w1b[:, :H] = W1[D:2 * D].astype(ml_dtypes.bfloat16)
    b1p = np.zeros((HP,), np.float32)
    b1p[:H] = b1
    wlp = np.zeros((HP,), np.float32)
    wlp[:H] = W1[2 * D]
    w2p = np.zeros((HP, L), np.float32)
    w2p[:H] = W2

    in_maps = []
    for c in range(N_CORES):
        b, p = c // 2, c % 2
        meta = np.zeros((1, 8), np.float32)
        meta[0, 0] = float(pred_spans[b, 0])
        meta[0, 1] = float(pred_spans[b, 1])
        meta[0, 2] = float(p)
        in_maps.append({
            "vecs_full": np.ascontiguousarray(vecs[b]),
            "vecs_loc": np.ascontiguousarray(vecs[b, p::2]),
            "w1a": w1a, "w1b": w1b, "b1p": b1p, "wlp": wlp, "w2p": w2p,
            "b2": b2,
            "avail": np.ascontiguousarray(span_avail[p::2]),
            "meta": meta,
        })
    return in_maps


def unshard(results):
    """results: list of 8 dicts with 'out' [IH*T, L] -> full [B, T*T, L]."""
    full = np.empty((B, T, T, L), np.float32)
    for c in range(N_CORES):
        b, p = c // 2, c % 2
        full[b, p::2] = results[c]["out"].reshape(IH, T, L)
    return full.reshape(B, T * T, L)


def kernel(hidden, W1, b1, W2, b2, pred_spans, span_avail, token_num):
    assert int(np.asarray(token_num)) == T, "kernel specialized for T=128"
    in_maps = make_in_maps(hidden, W1, b1, W2, b2, pred_spans, span_avail)
    nc = _get_program()
    res = bass_utils.run_bass_kernel_spmd(
        nc, in_maps, core_ids=list(range(N_CORES)))
    return unshard(res.results)
